# revision 12
# baseline (speedup 1.0000x reference)
"""Trainium2 Bass kernel: 2-layer LSTM (B=1024, T=512, H=256) + linear head.

Data-parallel across 8 NeuronCores: each core runs the full sequential scan
for a 128-row batch shard. Host-side work is marshaling only: sharding,
weight transposes/permutation (DoubleRow interleave), folding the
day-embedding into layer-0 input weights, and one-hot encoding the day column.

v2: fp8e4 DoubleRow matmuls for the three K=256 recurrent weight groups
(weights pre-scaled x16 host-side, descaled for free via ACT scale=1/16),
fp8 hidden state + fp8 PE transposes, GPSIMD offload of layer-1 cell-update
multiplies, layer-0 critical path kept short (split sigmoid, early tanh-c).
"""

import sys

import numpy as np

try:
    import ml_dtypes
    _BF16 = ml_dtypes.bfloat16
except ImportError:
    _BF16 = None

try:
    import concourse.bass as _probe  # noqa: F401
except ImportError:
    sys.path.insert(0, "/opt/trn_rl_repo")

B_FULL, T, D, H, P_OUT = 1024, 512, 64, 256, 14
N_CORES = 8
B = B_FULL // N_CORES  # 128 rows per core
G = 4 * H  # 1024 gate width
FA = 16  # augmented input rows: [val, onehot(day) x7, ones, pad x7]
CH = 64  # timesteps per aug SBUF chunk
NCH = T // CH
WS = 1.0  # no pre-scale needed for bf16

_PERM = np.concatenate(
    [np.arange(0, 512), np.arange(768, 1024), np.arange(512, 768)]
)

_MODULE = None
LAST_RESULTS = None


def _build_module():
    from contextlib import ExitStack

    import concourse.mybir as mybir
    from concourse import bacc
    from concourse.masks import make_identity
    from concourse.tile import TileContext

    f32 = mybir.dt.float32
    f32r = mybir.dt.float32r
    bf16 = mybir.dt.bfloat16
    fp8 = mybir.dt.float8e4
    Sig = mybir.ActivationFunctionType.Sigmoid
    Tanh = mybir.ActivationFunctionType.Tanh
    DR = mybir.MatmulPerfMode.DoubleRow
    SINV = 1.0

    nc = bacc.Bacc()
    aug_d = nc.dram_tensor("aug", [FA, T * B], bf16, kind="ExternalInput")
    z112_d = nc.dram_tensor("z112", [128 - FA, CH * B], bf16, kind="ExternalInput")
    w0t_d = nc.dram_tensor("w0t", [128, G], bf16, kind="ExternalInput")
    # DoubleRow-interleaved [128, 2*G] f32 staging; cast to fp8e4 on device.
    whh0dr_d = nc.dram_tensor("whh0dr", [128, 2 * G], f32, kind="ExternalInput")
    wih1dr_d = nc.dram_tensor("wih1dr", [128, 2 * G], f32, kind="ExternalInput")
    whh1dr_d = nc.dram_tensor("whh1dr", [128, 2 * G], f32, kind="ExternalInput")
    e0_d = nc.dram_tensor("e0", [128, 128], bf16, kind="ExternalInput")
    b1f_d = nc.dram_tensor("b1f", [128, G], bf16, kind="ExternalInput")
    wlint_d = nc.dram_tensor("wlint", [H, P_OUT], f32r, kind="ExternalInput")
    blinf_d = nc.dram_tensor("blinf", [128, P_OUT], bf16, kind="ExternalInput")
    out_d = nc.dram_tensor("out", [B, P_OUT], f32, kind="ExternalOutput")

    with TileContext(nc) as tc, ExitStack() as ctx:
        consts = ctx.enter_context(tc.tile_pool(name="consts", bufs=1))
        h0Tp = ctx.enter_context(tc.tile_pool(name="h0Tp", bufs=3))
        h1Tp = ctx.enter_context(tc.tile_pool(name="h1Tp", bufs=3))
        c0p = ctx.enter_context(tc.tile_pool(name="c0p", bufs=2))
        c1p = ctx.enter_context(tc.tile_pool(name="c1p", bufs=2))
        acts = ctx.enter_context(tc.tile_pool(name="acts", bufs=2))
        g0pp = ctx.enter_context(tc.tile_pool(name="g0pp", bufs=2, space="PSUM"))
        g1pp = ctx.enter_context(tc.tile_pool(name="g1pp", bufs=1, space="PSUM"))
        hTps = ctx.enter_context(tc.tile_pool(name="hTps", bufs=2, space="PSUM"))

        # --- constants to SBUF ---
        w0t_sb = consts.tile([128, G], bf16, tag="w0t")
        nc.sync.dma_start(w0t_sb, w0t_d[:, :])
        # fp8 DoubleRow weights: DMA f32 staging, cast once on DVE.
        dr_sb = {}
        for name, dram in (
            ("whh0", whh0dr_d),
            ("wih1", wih1dr_d),
            ("whh1", whh1dr_d),
        ):
            stage = consts.tile([128, 2 * G], f32, tag=f"stage_{name}")
            nc.sync.dma_start(stage, dram[:, :])
            w8 = consts.tile([128, 2 * G], bf16, tag=f"dr_{name}")
            nc.vector.tensor_copy(w8, stage)
            dr_sb[name] = w8.rearrange("p (j n) -> p j n", j=2)
        e0_sb = consts.tile([128, 128], bf16, tag="e0")
        nc.sync.dma_start(e0_sb, e0_d[:, :])
        b1f_sb = consts.tile([128, G], bf16, tag="b1f")
        nc.sync.dma_start(b1f_sb, b1f_d[:, :])
        wlint_sb = consts.tile([128, 2 * P_OUT], f32r, tag="wlint")
        for k in range(2):
            nc.sync.dma_start(
                wlint_sb[:, k * P_OUT : (k + 1) * P_OUT],
                wlint_d[k * 128 : (k + 1) * 128, :],
            )
        blinf_sb = consts.tile([128, P_OUT], bf16, tag="blinf")
        nc.sync.dma_start(blinf_sb, blinf_d[:, :])
        identb = consts.tile([128, 128], bf16, tag="identb")
        make_identity(nc, identb)
        identf = consts.tile([128, 128], f32, tag="identf")
        make_identity(nc, identf)

        # Two persistent aug buffers (manual double-buffer). Rows FA:128 are
        # zeroed once so the aug matmul can run with K=128.
        aug_bufs = []
        for i in range(2):
            ab = consts.tile([128, CH * B], bf16, tag=f"augbuf{i}", name=f"augbuf{i}")
            nc.sync.dma_start(ab[FA:128, :], z112_d[:, :])
            aug_bufs.append(ab)

        def load_chunk(chi):
            nc.sync.dma_start(
                aug_bufs[chi % 2][0:FA, :],
                aug_d[:, chi * CH * B : (chi + 1) * CH * B],
            )

        load_chunk(0)
        load_chunk(1)

        mm = nc.tensor.matmul

        h0T = [None] * T  # fp8 [128, 256] = [Hslice(part), j, B] flattened
        h1T = [None] * T
        c0 = [None] * T
        c1 = [None] * T
        h0n = [None] * T
        h1n = [None] * T
        sig = [[None] * T, [None] * T]
        gt = [[None] * T, [None] * T]
        g0ps = [None] * T
        g1ps = [None] * T
        h0tps = [None] * T
        h1tps = [None] * T
        sigo0 = [None] * T
        tc0x = [None] * T
        tc1x = [None] * T

        bk = [slice(0, 512), slice(512, 1024)]

        def drview(t8):
            return t8.rearrange("p (j m) -> p j m", j=2)

        def emit_g0_mms(t):
            chi = t // CH
            if t % CH == 0:
                if chi + 2 < NCH:
                    load_chunk(chi + 2)
            aug_sl = aug_bufs[chi % 2][:, (t % CH) * B : (t % CH + 1) * B]
            g0 = g0pp.tile([B, G], f32, tag="g0", name=f"g0_{t}")
            g0ps[t] = g0
            if t == 0:
                for nb in range(2):
                    mm(g0[:, bk[nb]], aug_sl[0:FA, :], w0t_sb[0:FA, bk[nb]],
                       start=True, stop=True, tile_position=(0, 0))
                return
            for nb in range(2):
                mm(g0[:, bk[nb]], aug_sl[0:FA, :], w0t_sb[0:FA, bk[nb]],
                   start=True, stop=False, tile_position=(0, 0))
            hp = h0T[t - 1]
            # bank0 group completes first so sigmoid_if starts earlier
            for nb in range(2):
                for k in range(2):
                    mm(
                        g0[:, bk[nb]],
                        hp[:, k * 128 : (k + 1) * 128],
                        dr_sb["whh0"][:, k, nb * 512 : (nb + 1) * 512],
                        start=False,
                        stop=(k == 1),
                    )

        def emit_g1_bias_ih1(t):
            g1 = g1pp.tile([B, G], f32, tag="g1", name=f"g1_{t}")
            g1ps[t] = g1
            chi = t // CH
            ones_sl = aug_bufs[chi % 2][32:33, (t % CH) * B : (t % CH + 1) * B]
            for nb in range(2):
                mm(g1[:, bk[nb]], ones_sl, b1f_sb[32:33, bk[nb]],
                   start=True, stop=False, tile_position=(32, 0))
            hp = h0T[t]
            for nb in range(2):
                for k in range(2):
                    mm(
                        g1[:, bk[nb]],
                        hp[:, k * 128 : (k + 1) * 128],
                        dr_sb["wih1"][:, k, nb * 512 : (nb + 1) * 512],
                        start=False,
                        stop=(t == 0 and k == 1),
                    )

        def emit_g1_hh1(t):
            g1 = g1ps[t]
            hq = h1T[t - 1]
            for nb in range(2):
                for k in range(2):
                    mm(
                        g1[:, bk[nb]],
                        hq[:, k * 128 : (k + 1) * 128],
                        dr_sb["whh1"][:, k, nb * 512 : (nb + 1) * 512],
                        start=False,
                        stop=(k == 1),
                    )

        def emit_chain_a0(t):
            gps = g0ps[t]
            s = acts.tile([B, 2 * H], bf16, tag="sigif0", name=f"sigif0_{t}")
            sig[0][t] = s
            nc.scalar.activation(s, gps[:, 0 : 2 * H], Sig, scale=SINV)
            g = acts.tile([B, H], bf16, tag="gt0", name=f"gt0_{t}")
            gt[0][t] = g
            nc.scalar.activation(g, gps[:, 3 * H : G], Tanh, scale=SINV)

        def emit_sigo0(t):
            gps = g0ps[t]
            so = acts.tile([B, H], bf16, tag="sigo0", name=f"sigo0_{t}")
            sigo0[t] = so
            nc.scalar.activation(so, gps[:, 2 * H : 3 * H], Sig, scale=SINV)

        def emit_chain_a1(t):
            gps = g1ps[t]
            s = acts.tile([B, 3 * H], bf16, tag="sig1", name=f"sig1_{t}")
            sig[1][t] = s
            nc.scalar.activation(s, gps[:, 0 : 3 * H], Sig, scale=SINV)
            g = acts.tile([B, H], bf16, tag="gt1", name=f"gt1_{t}")
            gt[1][t] = g
            nc.scalar.activation(g, gps[:, 3 * H : G], Tanh, scale=SINV)

        def emit_cadd(layer, t):
            # c = f*c + i*g ; layer 1 multiplies on GPSIMD (off critical path)
            cp = c0p if layer == 0 else c1p
            cl = c0 if layer == 0 else c1
            feng = nc.vector if layer == 0 else nc.gpsimd
            s = sig[layer][t]
            g = gt[layer][t]
            cn = cp.tile([B, H], bf16, tag=f"c{layer}", name=f"c{layer}_{t}")
            if t == 0:
                nc.vector.tensor_mul(cn, s[:, 0:H], g)
            else:
                fc = acts.tile([B, H], bf16, tag=f"fc{layer}", name=f"fc{layer}_{t}")
                feng.tensor_mul(fc, s[:, H : 2 * H], cl[t - 1])
                ig = acts.tile([B, H], bf16, tag=f"ig{layer}", name=f"ig{layer}_{t}")
                nc.vector.tensor_mul(ig, s[:, 0:H], g)
                nc.vector.tensor_add(cn, ig, fc)
            cl[t] = cn

        def emit_tanhc(layer, t):
            cl = c0 if layer == 0 else c1
            tcx = acts.tile([B, H], bf16, tag=f"tc{layer}", name=f"tc{layer}_{t}")
            nc.scalar.activation(tcx, cl[t], Tanh)
            if layer == 0:
                tc0x[t] = tcx
            else:
                tc1x[t] = tcx

        def emit_h(layer, t, final=False):
            hn = h0n if layer == 0 else h1n
            so = sigo0[t] if layer == 0 else sig[1][t][:, 2 * H : 3 * H]
            tcx = tc0x[t] if layer == 0 else tc1x[t]
            dt = f32 if final else bf16
            h = acts.tile([B, H], dt, tag=f"hn{layer}{'f' if final else ''}",
                          name=f"hn{layer}_{t}")
            nc.vector.tensor_mul(h, so, tcx)
            hn[t] = h

        def emit_hT(layer, t):
            hn = h0n if layer == 0 else h1n
            hTl = h0T if layer == 0 else h1T
            hTpool = h0Tp if layer == 0 else h1Tp
            tps = h0tps if layer == 0 else h1tps
            ps = hTps.tile([128, 256], bf16, tag="htp", name=f"h{layer}tp_{t}")
            nc.tensor.transpose(ps[:, 0:128], hn[t][:, 0:128], identb)
            nc.tensor.transpose(ps[:, 128:256], hn[t][:, 128:256], identb)
            tps[t] = ps
            hsb = hTpool.tile([128, 2 * 128], bf16, tag=f"h{layer}T", name=f"h{layer}T_{t}")
            nc.vector.tensor_copy(
                hsb.bitcast(mybir.dt.uint32), ps.bitcast(mybir.dt.uint32)
            )
            hTl[t] = hsb

        for tau in range(T + 2):
            t0 = tau  # layer-0 step handled this tick
            t1 = tau - 1  # layer-1 step handled this tick
            do0 = t0 < T
            do1 = 0 <= t1 < T
            if do0:
                emit_g0_mms(t0)
                emit_chain_a0(t0)
            if do1:
                emit_g1_bias_ih1(t1)
                if t1 >= 1:
                    emit_g1_hh1(t1)
            if do0:
                emit_sigo0(t0)
            if do1:
                emit_chain_a1(t1)
            if do0:
                emit_cadd(0, t0)
            if do1:
                emit_cadd(1, t1)
            if do0:
                emit_tanhc(0, t0)
            if do1:
                emit_tanhc(1, t1)
            if do0:
                emit_h(0, t0)
                emit_hT(0, t0)
            if do1:
                final = t1 == T - 1
                emit_h(1, t1, final=final)
                if not final:
                    emit_hT(1, t1)

        # ------------- final linear: out = h1[T-1] @ Wlin.T + blin -------------
        # h1[T-1] was produced in f32; transpose in f32 for full precision.
        psf = hTps.tile([128, 512], f32, tag="htp", name="hlastT")
        nc.tensor.transpose(psf[:, 0:128], h1n[T - 1][:, 0:128], identf)
        nc.tensor.transpose(psf[:, 256:384], h1n[T - 1][:, 128:256], identf)
        hl = consts.tile([128, H], f32r, tag="hlT")
        nc.vector.tensor_copy(
            hl.rearrange("p (b c) -> p b c", b=2),
            psf.rearrange("p (b c) -> p b c", b=2)[:, :, 0:128],
        )
        outp = hTps.tile([B, P_OUT], f32, tag="htp", name="outp")
        mm(outp, e0_sb, blinf_sb, start=True, stop=False)
        for k in range(2):
            mm(
                outp,
                hl[:, k * 128 : (k + 1) * 128],
                wlint_sb[:, k * P_OUT : (k + 1) * P_OUT],
                start=False,
                stop=(k == 1),
            )
        out_sb = consts.tile([B, P_OUT], f32, tag="outsb")
        nc.vector.tensor_copy(out_sb, outp)
        nc.sync.dma_start(out_d[:, :], out_sb)

    nc.finalize()
    return nc


def _get_module():
    global _MODULE
    if _MODULE is None:
        _MODULE = _build_module()
    return _MODULE


def _dr_layout(w_perm_t):
    # w_perm_t: [H=256, G] (already W[_PERM].T). Returns [128, 2*G] with the
    # two K-chunks side by side: element [p, j*G + n] = w_perm_t[j*128 + p, n].
    return np.ascontiguousarray(
        w_perm_t.reshape(2, 128, G).transpose(1, 0, 2).reshape(128, 2 * G)
    )


def kernel(**inputs):
    global LAST_RESULTS
    from concourse.bass_utils import run_bass_kernel_spmd

    f = lambda a: np.ascontiguousarray(np.asarray(a), dtype=np.float32)
    x = f(inputs["x"])
    emb = f(inputs["emb"])
    Wih0, Whh0 = f(inputs["Wih0"]), f(inputs["Whh0"])
    bih0, bhh0 = f(inputs["bih0"]), f(inputs["bhh0"])
    Wih1, Whh1 = f(inputs["Wih1"]), f(inputs["Whh1"])
    bih1, bhh1 = f(inputs["bih1"]), f(inputs["bhh1"])
    Wlin, blin = f(inputs["Wlin"]), f(inputs["blin"])

    # Fold embedding + biases into layer-0 input weights (scaled by WS).
    w_val = Wih0[:, 0:1]  # [G, 1]
    M0 = Wih0[:, 1 : 1 + D] @ emb.T  # [G, 7]
    b0 = (bih0 + bhh0)[:, None]  # [G, 1]
    W0aug = np.concatenate(
        [w_val, M0, b0, np.zeros((G, 128 - 9), np.float32)], axis=1
)  # [G, 128]

    w0t = np.ascontiguousarray(W0aug[_PERM].T)  # [128, G]
    whh0dr = _dr_layout(np.ascontiguousarray(Whh0[_PERM].T) * WS)
    wih1dr = _dr_layout(np.ascontiguousarray(Wih1[_PERM].T) * WS)
    whh1dr = _dr_layout(np.ascontiguousarray(Whh1[_PERM].T) * WS)
    b1f = np.zeros((128, G), np.float32)
    b1f[32] = (bih1 + bhh1)[_PERM]
    e0 = np.zeros((128, 128), np.float32)
    e0[0] = 1.0
    wlint = np.ascontiguousarray(Wlin.T)  # [H, P_OUT]
    blinf = np.zeros((128, P_OUT), np.float32)
    blinf[0] = blin
    z112 = np.zeros((128 - FA, CH * B), np.float32)
    z112[32 - FA] = 1.0  # ones row at partition 32 for the bias matmul

    val = x[:, :, 0]  # [B_FULL, T]
    day = x[:, :, 1].astype(np.int32)  # [B_FULL, T]

    in_maps = []
    for c in range(N_CORES):
        sl = slice(c * B, (c + 1) * B)
        aug = np.zeros((FA, T, B), np.float32)
        aug[0] = val[sl].T
        dT = day[sl].T  # [T, B]
        for d in range(7):
            aug[1 + d] = dT == d
        aug[8] = 1.0
        in_maps.append(
            {
                "aug": np.ascontiguousarray(aug.reshape(FA, T * B)).astype(_BF16),
                "z112": z112.astype(_BF16),
                "w0t": w0t.astype(_BF16),
                "whh0dr": whh0dr,
                "wih1dr": wih1dr,
                "whh1dr": whh1dr,
                "e0": e0.astype(_BF16),
                "b1f": b1f.astype(_BF16),
                "wlint": wlint,
                "blinf": blinf.astype(_BF16),
            }
        )

    res = run_bass_kernel_spmd(_get_module(), in_maps, core_ids=list(range(N_CORES)))
    LAST_RESULTS = res
    out = np.concatenate([r["out"] for r in res.results], axis=0)
    return np.ascontiguousarray(out, dtype=np.float32)


# revision 13
# speedup vs baseline: 1.0211x; 1.0211x over previous
"""Trainium2 Bass kernel: 2-layer LSTM (B=1024, T=512, H=256) + linear head.

Data-parallel across 8 NeuronCores: each core runs the full sequential scan
for a 128-row batch shard. Host-side work is marshaling only: sharding,
weight transposes/permutation (DoubleRow interleave), folding the
day-embedding into layer-0 input weights, and one-hot encoding the day column.

v2: fp8e4 DoubleRow matmuls for the three K=256 recurrent weight groups
(weights pre-scaled x16 host-side, descaled for free via ACT scale=1/16),
fp8 hidden state + fp8 PE transposes, GPSIMD offload of layer-1 cell-update
multiplies, layer-0 critical path kept short (split sigmoid, early tanh-c).
"""

import sys

import numpy as np

try:
    import ml_dtypes
    _BF16 = ml_dtypes.bfloat16
except ImportError:
    _BF16 = None

try:
    import concourse.bass as _probe  # noqa: F401
except ImportError:
    sys.path.insert(0, "/opt/trn_rl_repo")

B_FULL, T, D, H, P_OUT = 1024, 512, 64, 256, 14
N_CORES = 8
B = B_FULL // N_CORES  # 128 rows per core
G = 4 * H  # 1024 gate width
FA = 16  # augmented input rows: [val, onehot(day) x7, ones, pad x7]
CH = 64  # timesteps per aug SBUF chunk
NCH = T // CH
WS = 1.0  # no pre-scale needed for bf16

_PERM = np.concatenate(
    [np.arange(0, 512), np.arange(768, 1024), np.arange(512, 768)]
)

_MODULE = None
LAST_RESULTS = None


def _build_module():
    from contextlib import ExitStack

    import concourse.mybir as mybir
    from concourse import bacc
    from concourse.masks import make_identity
    from concourse.tile import TileContext

    f32 = mybir.dt.float32
    f32r = mybir.dt.float32r
    bf16 = mybir.dt.bfloat16
    fp8 = mybir.dt.float8e4
    Sig = mybir.ActivationFunctionType.Sigmoid
    Tanh = mybir.ActivationFunctionType.Tanh
    DR = mybir.MatmulPerfMode.DoubleRow
    SINV = 1.0

    nc = bacc.Bacc()
    aug_d = nc.dram_tensor("aug", [FA, T * B], bf16, kind="ExternalInput")
    z112_d = nc.dram_tensor("z112", [128 - FA, CH * B], bf16, kind="ExternalInput")
    w0t_d = nc.dram_tensor("w0t", [128, G], bf16, kind="ExternalInput")
    # DoubleRow-interleaved [128, 2*G] f32 staging; cast to fp8e4 on device.
    whh0dr_d = nc.dram_tensor("whh0dr", [128, 2 * G], f32, kind="ExternalInput")
    wih1dr_d = nc.dram_tensor("wih1dr", [128, 2 * G], f32, kind="ExternalInput")
    whh1dr_d = nc.dram_tensor("whh1dr", [128, 2 * G], f32, kind="ExternalInput")
    e0_d = nc.dram_tensor("e0", [128, 128], bf16, kind="ExternalInput")
    b1f_d = nc.dram_tensor("b1f", [128, G], bf16, kind="ExternalInput")
    wlint_d = nc.dram_tensor("wlint", [H, P_OUT], f32r, kind="ExternalInput")
    blinf_d = nc.dram_tensor("blinf", [128, P_OUT], bf16, kind="ExternalInput")
    out_d = nc.dram_tensor("out", [B, P_OUT], f32, kind="ExternalOutput")

    with TileContext(nc) as tc, ExitStack() as ctx:
        consts = ctx.enter_context(tc.tile_pool(name="consts", bufs=1))
        h0Tp = ctx.enter_context(tc.tile_pool(name="h0Tp", bufs=3))
        h1Tp = ctx.enter_context(tc.tile_pool(name="h1Tp", bufs=3))
        c0p = ctx.enter_context(tc.tile_pool(name="c0p", bufs=2))
        c1p = ctx.enter_context(tc.tile_pool(name="c1p", bufs=2))
        acts = ctx.enter_context(tc.tile_pool(name="acts", bufs=2))
        g0pp = ctx.enter_context(tc.tile_pool(name="g0pp", bufs=2, space="PSUM"))
        g1pp = ctx.enter_context(tc.tile_pool(name="g1pp", bufs=1, space="PSUM"))
        hTps = ctx.enter_context(tc.tile_pool(name="hTps", bufs=2, space="PSUM"))

        # --- constants to SBUF ---
        w0t_sb = consts.tile([128, G], bf16, tag="w0t")
        nc.sync.dma_start(w0t_sb, w0t_d[:, :])
        # fp8 DoubleRow weights: DMA f32 staging, cast once on DVE.
        dr_sb = {}
        for name, dram in (
            ("whh0", whh0dr_d),
            ("wih1", wih1dr_d),
            ("whh1", whh1dr_d),
        ):
            stage = consts.tile([128, 2 * G], f32, tag=f"stage_{name}")
            nc.sync.dma_start(stage, dram[:, :])
            w8 = consts.tile([128, 2 * G], bf16, tag=f"dr_{name}")
            nc.vector.tensor_copy(w8, stage)
            dr_sb[name] = w8.rearrange("p (j n) -> p j n", j=2)
        e0_sb = consts.tile([128, 128], bf16, tag="e0")
        nc.sync.dma_start(e0_sb, e0_d[:, :])
        b1f_sb = consts.tile([128, G], bf16, tag="b1f")
        nc.sync.dma_start(b1f_sb, b1f_d[:, :])
        wlint_sb = consts.tile([128, 2 * P_OUT], f32r, tag="wlint")
        for k in range(2):
            nc.sync.dma_start(
                wlint_sb[:, k * P_OUT : (k + 1) * P_OUT],
                wlint_d[k * 128 : (k + 1) * 128, :],
            )
        blinf_sb = consts.tile([128, P_OUT], bf16, tag="blinf")
        nc.sync.dma_start(blinf_sb, blinf_d[:, :])
        identb = consts.tile([128, 128], bf16, tag="identb")
        make_identity(nc, identb)
        identf = consts.tile([128, 128], f32, tag="identf")
        make_identity(nc, identf)

        # Two persistent aug buffers (manual double-buffer). Rows FA:128 are
        # zeroed once so the aug matmul can run with K=128.
        aug_bufs = []
        for i in range(2):
            ab = consts.tile([128, CH * B], bf16, tag=f"augbuf{i}", name=f"augbuf{i}")
            nc.sync.dma_start(ab[FA:128, :], z112_d[:, :])
            aug_bufs.append(ab)

        def load_chunk(chi):
            nc.sync.dma_start(
                aug_bufs[chi % 2][0:FA, :],
                aug_d[:, chi * CH * B : (chi + 1) * CH * B],
            )

        load_chunk(0)
        load_chunk(1)

        mm = nc.tensor.matmul

        h0T = [None] * T  # fp8 [128, 256] = [Hslice(part), j, B] flattened
        h1T = [None] * T
        c0 = [None] * T
        c1 = [None] * T
        h0n = [None] * T
        h1n = [None] * T
        sig = [[None] * T, [None] * T]
        gt = [[None] * T, [None] * T]
        g0ps = [None] * T
        g1ps = [None] * T
        h0tps = [None] * T
        h1tps = [None] * T
        sigo0 = [None] * T
        tc0x = [None] * T
        tc1x = [None] * T

        bk = [slice(0, 512), slice(512, 1024)]

        def drview(t8):
            return t8.rearrange("p (j m) -> p j m", j=2)

        def emit_g0_mms(t):
            chi = t // CH
            if t % CH == 0:
                if chi + 2 < NCH:
                    load_chunk(chi + 2)
            aug_sl = aug_bufs[chi % 2][:, (t % CH) * B : (t % CH + 1) * B]
            g0 = g0pp.tile([B, G], f32, tag="g0", name=f"g0_{t}")
            g0ps[t] = g0
            if t == 0:
                for nb in range(2):
                    mm(g0[:, bk[nb]], aug_sl, w0t_sb[:, bk[nb]], start=True, stop=True)
                return
            for nb in range(2):
                mm(g0[:, bk[nb]], aug_sl, w0t_sb[:, bk[nb]], start=True, stop=False)
            hp = h0T[t - 1]
            # bank0 group completes first so sigmoid_if starts earlier
            for nb in range(2):
                for k in range(2):
                    mm(
                        g0[:, bk[nb]],
                        hp[:, k * 128 : (k + 1) * 128],
                        dr_sb["whh0"][:, k, nb * 512 : (nb + 1) * 512],
                        start=False,
                        stop=(k == 1),
                    )

        def emit_g1_bias_ih1(t):
            g1 = g1pp.tile([B, G], f32, tag="g1", name=f"g1_{t}")
            g1ps[t] = g1
            for nb in range(2):
                mm(g1[:, bk[nb]], e0_sb, b1f_sb[:, bk[nb]], start=True, stop=False)
            hp = h0T[t]
            for nb in range(2):
                for k in range(2):
                    mm(
                        g1[:, bk[nb]],
                        hp[:, k * 128 : (k + 1) * 128],
                        dr_sb["wih1"][:, k, nb * 512 : (nb + 1) * 512],
                        start=False,
                        stop=(t == 0 and k == 1),
                    )

        def emit_g1_hh1(t):
            g1 = g1ps[t]
            hq = h1T[t - 1]
            for nb in range(2):
                for k in range(2):
                    mm(
                        g1[:, bk[nb]],
                        hq[:, k * 128 : (k + 1) * 128],
                        dr_sb["whh1"][:, k, nb * 512 : (nb + 1) * 512],
                        start=False,
                        stop=(k == 1),
                    )

        def emit_chain_a0(t):
            gps = g0ps[t]
            s = acts.tile([B, 2 * H], bf16, tag="sigif0", name=f"sigif0_{t}")
            sig[0][t] = s
            nc.scalar.activation(s, gps[:, 0 : 2 * H], Sig, scale=SINV)
            g = acts.tile([B, H], bf16, tag="gt0", name=f"gt0_{t}")
            gt[0][t] = g
            nc.scalar.activation(g, gps[:, 3 * H : G], Tanh, scale=SINV)

        def emit_sigo0(t):
            gps = g0ps[t]
            so = acts.tile([B, H], bf16, tag="sigo0", name=f"sigo0_{t}")
            sigo0[t] = so
            nc.scalar.activation(so, gps[:, 2 * H : 3 * H], Sig, scale=SINV)

        def emit_chain_a1(t):
            gps = g1ps[t]
            s = acts.tile([B, 3 * H], bf16, tag="sig1", name=f"sig1_{t}")
            sig[1][t] = s
            nc.scalar.activation(s, gps[:, 0 : 3 * H], Sig, scale=SINV)
            g = acts.tile([B, H], bf16, tag="gt1", name=f"gt1_{t}")
            gt[1][t] = g
            nc.scalar.activation(g, gps[:, 3 * H : G], Tanh, scale=SINV)

        def emit_cadd(layer, t):
            # c = f*c + i*g ; layer 1 multiplies on GPSIMD (off critical path)
            cp = c0p if layer == 0 else c1p
            cl = c0 if layer == 0 else c1
            feng = nc.vector if layer == 0 else nc.gpsimd
            s = sig[layer][t]
            g = gt[layer][t]
            cn = cp.tile([B, H], bf16, tag=f"c{layer}", name=f"c{layer}_{t}")
            if t == 0:
                nc.vector.tensor_mul(cn, s[:, 0:H], g)
            else:
                fc = acts.tile([B, H], bf16, tag=f"fc{layer}", name=f"fc{layer}_{t}")
                feng.tensor_mul(fc, s[:, H : 2 * H], cl[t - 1])
                ig = acts.tile([B, H], bf16, tag=f"ig{layer}", name=f"ig{layer}_{t}")
                nc.vector.tensor_mul(ig, s[:, 0:H], g)
                nc.vector.tensor_add(cn, ig, fc)
            cl[t] = cn

        def emit_tanhc(layer, t):
            cl = c0 if layer == 0 else c1
            tcx = acts.tile([B, H], bf16, tag=f"tc{layer}", name=f"tc{layer}_{t}")
            nc.scalar.activation(tcx, cl[t], Tanh)
            if layer == 0:
                tc0x[t] = tcx
            else:
                tc1x[t] = tcx

        def emit_h(layer, t, final=False):
            hn = h0n if layer == 0 else h1n
            so = sigo0[t] if layer == 0 else sig[1][t][:, 2 * H : 3 * H]
            tcx = tc0x[t] if layer == 0 else tc1x[t]
            dt = f32 if final else bf16
            h = acts.tile([B, H], dt, tag=f"hn{layer}{'f' if final else ''}",
                          name=f"hn{layer}_{t}")
            nc.vector.tensor_mul(h, so, tcx)
            hn[t] = h

        def emit_hT(layer, t):
            hn = h0n if layer == 0 else h1n
            hTl = h0T if layer == 0 else h1T
            hTpool = h0Tp if layer == 0 else h1Tp
            tps = h0tps if layer == 0 else h1tps
            ps = hTps.tile([128, 256], bf16, tag="htp", name=f"h{layer}tp_{t}")
            nc.tensor.transpose(ps[:, 0:128], hn[t][:, 0:128], identb)
            nc.tensor.transpose(ps[:, 128:256], hn[t][:, 128:256], identb)
            tps[t] = ps
            hsb = hTpool.tile([128, 2 * 128], bf16, tag=f"h{layer}T", name=f"h{layer}T_{t}")
            nc.vector.tensor_copy(
                hsb.bitcast(mybir.dt.uint32), ps.bitcast(mybir.dt.uint32)
            )
            hTl[t] = hsb

        for tau in range(T + 2):
            t0 = tau  # layer-0 step handled this tick
            t1 = tau - 1  # layer-1 step handled this tick
            do0 = t0 < T
            do1 = 0 <= t1 < T
            if do0:
                emit_g0_mms(t0)
                emit_chain_a0(t0)
            if do1:
                emit_g1_bias_ih1(t1)
                if t1 >= 1:
                    emit_g1_hh1(t1)
            if do0:
                emit_sigo0(t0)
            if do1:
                emit_chain_a1(t1)
            if do0:
                emit_cadd(0, t0)
            if do1:
                emit_cadd(1, t1)
            if do0:
                emit_tanhc(0, t0)
            if do1:
                emit_tanhc(1, t1)
            if do0:
                emit_h(0, t0)
                emit_hT(0, t0)
            if do1:
                final = t1 == T - 1
                emit_h(1, t1, final=final)
                if not final:
                    emit_hT(1, t1)

        # ------------- final linear: out = h1[T-1] @ Wlin.T + blin -------------
        # h1[T-1] was produced in f32; transpose in f32 for full precision.
        psf = hTps.tile([128, 512], f32, tag="htp", name="hlastT")
        nc.tensor.transpose(psf[:, 0:128], h1n[T - 1][:, 0:128], identf)
        nc.tensor.transpose(psf[:, 256:384], h1n[T - 1][:, 128:256], identf)
        hl = consts.tile([128, H], f32r, tag="hlT")
        nc.vector.tensor_copy(
            hl.rearrange("p (b c) -> p b c", b=2),
            psf.rearrange("p (b c) -> p b c", b=2)[:, :, 0:128],
        )
        outp = hTps.tile([B, P_OUT], f32, tag="htp", name="outp")
        mm(outp, e0_sb, blinf_sb, start=True, stop=False)
        for k in range(2):
            mm(
                outp,
                hl[:, k * 128 : (k + 1) * 128],
                wlint_sb[:, k * P_OUT : (k + 1) * P_OUT],
                start=False,
                stop=(k == 1),
            )
        out_sb = consts.tile([B, P_OUT], f32, tag="outsb")
        nc.vector.tensor_copy(out_sb, outp)
        nc.sync.dma_start(out_d[:, :], out_sb)

    nc.finalize()
    return nc


def _get_module():
    global _MODULE
    if _MODULE is None:
        _MODULE = _build_module()
    return _MODULE


def _dr_layout(w_perm_t):
    # w_perm_t: [H=256, G] (already W[_PERM].T). Returns [128, 2*G] with the
    # two K-chunks side by side: element [p, j*G + n] = w_perm_t[j*128 + p, n].
    return np.ascontiguousarray(
        w_perm_t.reshape(2, 128, G).transpose(1, 0, 2).reshape(128, 2 * G)
    )


def kernel(**inputs):
    global LAST_RESULTS
    from concourse.bass_utils import run_bass_kernel_spmd

    f = lambda a: np.ascontiguousarray(np.asarray(a), dtype=np.float32)
    x = f(inputs["x"])
    emb = f(inputs["emb"])
    Wih0, Whh0 = f(inputs["Wih0"]), f(inputs["Whh0"])
    bih0, bhh0 = f(inputs["bih0"]), f(inputs["bhh0"])
    Wih1, Whh1 = f(inputs["Wih1"]), f(inputs["Whh1"])
    bih1, bhh1 = f(inputs["bih1"]), f(inputs["bhh1"])
    Wlin, blin = f(inputs["Wlin"]), f(inputs["blin"])

    # Fold embedding + biases into layer-0 input weights (scaled by WS).
    w_val = Wih0[:, 0:1]  # [G, 1]
    M0 = Wih0[:, 1 : 1 + D] @ emb.T  # [G, 7]
    b0 = (bih0 + bhh0)[:, None]  # [G, 1]
    W0aug = np.concatenate(
        [w_val, M0, b0, np.zeros((G, 128 - 9), np.float32)], axis=1
)  # [G, 128]

    w0t = np.ascontiguousarray(W0aug[_PERM].T)  # [128, G]
    whh0dr = _dr_layout(np.ascontiguousarray(Whh0[_PERM].T) * WS)
    wih1dr = _dr_layout(np.ascontiguousarray(Wih1[_PERM].T) * WS)
    whh1dr = _dr_layout(np.ascontiguousarray(Whh1[_PERM].T) * WS)
    b1f = np.zeros((128, G), np.float32)
    b1f[0] = (bih1 + bhh1)[_PERM]
    e0 = np.zeros((128, 128), np.float32)
    e0[0] = 1.0
    wlint = np.ascontiguousarray(Wlin.T)  # [H, P_OUT]
    blinf = np.zeros((128, P_OUT), np.float32)
    blinf[0] = blin
    z112 = np.zeros((128 - FA, CH * B), np.float32)

    val = x[:, :, 0]  # [B_FULL, T]
    day = x[:, :, 1].astype(np.int32)  # [B_FULL, T]

    in_maps = []
    for c in range(N_CORES):
        sl = slice(c * B, (c + 1) * B)
        aug = np.zeros((FA, T, B), np.float32)
        aug[0] = val[sl].T
        dT = day[sl].T  # [T, B]
        for d in range(7):
            aug[1 + d] = dT == d
        aug[8] = 1.0
        in_maps.append(
            {
                "aug": np.ascontiguousarray(aug.reshape(FA, T * B)).astype(_BF16),
                "z112": z112.astype(_BF16),
                "w0t": w0t.astype(_BF16),
                "whh0dr": whh0dr,
                "wih1dr": wih1dr,
                "whh1dr": whh1dr,
                "e0": e0.astype(_BF16),
                "b1f": b1f.astype(_BF16),
                "wlint": wlint,
                "blinf": blinf.astype(_BF16),
            }
        )

    res = run_bass_kernel_spmd(_get_module(), in_maps, core_ids=list(range(N_CORES)))
    LAST_RESULTS = res
    out = np.concatenate([r["out"] for r in res.results], axis=0)
    return np.ascontiguousarray(out, dtype=np.float32)


# revision 15
# speedup vs baseline: 1.0212x; 1.0000x over previous
"""Trainium2 Bass kernel: 2-layer LSTM (B=1024, T=512, H=256) + linear head.

Data-parallel across 8 NeuronCores: each core runs the full sequential scan
for a 128-row batch shard. Host-side work is marshaling only: sharding,
weight transposes/permutation (DoubleRow interleave), folding the
day-embedding into layer-0 input weights, and one-hot encoding the day column.

Optimized vs the f32r original: all matmuls in bf16 (moving-operand streams
at 1 col/cycle regardless of dtype, but bf16 halves LDWEIGHTS via FWL and
enables wide moving tiles), bf16 cell state (2x DVE tensor ops), uint32-
bitcast hT copies, GPSIMD offload of the off-critical-path layer-1 f*c
multiply, double-buffered g0 PSUM (no aug-WAR stall), and a tick schedule
that fills the ACT idle hole with layer-1 sigmoids while keeping layer-0's
recurrence chain (split sigmoid -> tanh(g) -> cell update -> tanh(c) ->
h -> transpose -> copy -> whh0 matmul) as short as possible.
"""

import sys

import numpy as np

try:
    import ml_dtypes
    _BF16 = ml_dtypes.bfloat16
except ImportError:
    _BF16 = None

try:
    import concourse.bass as _probe  # noqa: F401
except ImportError:
    sys.path.insert(0, "/opt/trn_rl_repo")

B_FULL, T, D, H, P_OUT = 1024, 512, 64, 256, 14
N_CORES = 8
B = B_FULL // N_CORES  # 128 rows per core
G = 4 * H  # 1024 gate width
FA = 16  # augmented input rows: [val, onehot(day) x7, ones, pad x7]
CH = 64  # timesteps per aug SBUF chunk
NCH = T // CH
WS = 1.0  # no pre-scale needed for bf16

_PERM = np.concatenate(
    [np.arange(0, 512), np.arange(768, 1024), np.arange(512, 768)]
)

_MODULE = None
LAST_RESULTS = None


def _build_module():
    from contextlib import ExitStack

    import concourse.mybir as mybir
    from concourse import bacc
    from concourse.masks import make_identity
    from concourse.tile import TileContext

    f32 = mybir.dt.float32
    f32r = mybir.dt.float32r
    bf16 = mybir.dt.bfloat16
    fp8 = mybir.dt.float8e4
    Sig = mybir.ActivationFunctionType.Sigmoid
    Tanh = mybir.ActivationFunctionType.Tanh
    DR = mybir.MatmulPerfMode.DoubleRow
    SINV = 1.0

    nc = bacc.Bacc()
    aug_d = nc.dram_tensor("aug", [FA, T * B], bf16, kind="ExternalInput")
    z112_d = nc.dram_tensor("z112", [128 - FA, CH * B], bf16, kind="ExternalInput")
    w0t_d = nc.dram_tensor("w0t", [128, G], bf16, kind="ExternalInput")
    # DoubleRow-interleaved [128, 2*G] f32 staging; cast to fp8e4 on device.
    whh0dr_d = nc.dram_tensor("whh0dr", [128, 2 * G], f32, kind="ExternalInput")
    wih1dr_d = nc.dram_tensor("wih1dr", [128, 2 * G], f32, kind="ExternalInput")
    whh1dr_d = nc.dram_tensor("whh1dr", [128, 2 * G], f32, kind="ExternalInput")
    e0_d = nc.dram_tensor("e0", [128, 128], bf16, kind="ExternalInput")
    b1f_d = nc.dram_tensor("b1f", [128, G], bf16, kind="ExternalInput")
    wlint_d = nc.dram_tensor("wlint", [H, P_OUT], f32r, kind="ExternalInput")
    blinf_d = nc.dram_tensor("blinf", [128, P_OUT], bf16, kind="ExternalInput")
    out_d = nc.dram_tensor("out", [B, P_OUT], f32, kind="ExternalOutput")

    with TileContext(nc) as tc, ExitStack() as ctx:
        consts = ctx.enter_context(tc.tile_pool(name="consts", bufs=1))
        h0Tp = ctx.enter_context(tc.tile_pool(name="h0Tp", bufs=3))
        h1Tp = ctx.enter_context(tc.tile_pool(name="h1Tp", bufs=3))
        c0p = ctx.enter_context(tc.tile_pool(name="c0p", bufs=2))
        c1p = ctx.enter_context(tc.tile_pool(name="c1p", bufs=2))
        acts = ctx.enter_context(tc.tile_pool(name="acts", bufs=2))
        g0pp = ctx.enter_context(tc.tile_pool(name="g0pp", bufs=2, space="PSUM"))
        g1pp = ctx.enter_context(tc.tile_pool(name="g1pp", bufs=1, space="PSUM"))
        hTps = ctx.enter_context(tc.tile_pool(name="hTps", bufs=2, space="PSUM"))

        # --- constants to SBUF ---
        w0t_sb = consts.tile([128, G], bf16, tag="w0t")
        nc.sync.dma_start(w0t_sb, w0t_d[:, :])
        # fp8 DoubleRow weights: DMA f32 staging, cast once on DVE.
        dr_sb = {}
        for name, dram in (
            ("whh0", whh0dr_d),
            ("wih1", wih1dr_d),
            ("whh1", whh1dr_d),
        ):
            stage = consts.tile([128, 2 * G], f32, tag=f"stage_{name}")
            nc.sync.dma_start(stage, dram[:, :])
            w8 = consts.tile([128, 2 * G], bf16, tag=f"dr_{name}")
            nc.vector.tensor_copy(w8, stage)
            dr_sb[name] = w8.rearrange("p (j n) -> p j n", j=2)
        e0_sb = consts.tile([128, 128], bf16, tag="e0")
        nc.sync.dma_start(e0_sb, e0_d[:, :])
        b1f_sb = consts.tile([128, G], bf16, tag="b1f")
        nc.sync.dma_start(b1f_sb, b1f_d[:, :])
        wlint_sb = consts.tile([128, 2 * P_OUT], f32r, tag="wlint")
        for k in range(2):
            nc.sync.dma_start(
                wlint_sb[:, k * P_OUT : (k + 1) * P_OUT],
                wlint_d[k * 128 : (k + 1) * 128, :],
            )
        blinf_sb = consts.tile([128, P_OUT], bf16, tag="blinf")
        nc.sync.dma_start(blinf_sb, blinf_d[:, :])
        identb = consts.tile([128, 128], bf16, tag="identb")
        make_identity(nc, identb)
        identf = consts.tile([128, 128], f32, tag="identf")
        make_identity(nc, identf)

        # Two persistent aug buffers (manual double-buffer). Rows FA:128 are
        # zeroed once so the aug matmul can run with K=128.
        aug_bufs = []
        for i in range(2):
            ab = consts.tile([128, CH * B], bf16, tag=f"augbuf{i}", name=f"augbuf{i}")
            nc.sync.dma_start(ab[FA:128, :], z112_d[:, :])
            aug_bufs.append(ab)

        def load_chunk(chi):
            nc.sync.dma_start(
                aug_bufs[chi % 2][0:FA, :],
                aug_d[:, chi * CH * B : (chi + 1) * CH * B],
            )

        load_chunk(0)
        load_chunk(1)

        mm = nc.tensor.matmul

        h0T = [None] * T  # fp8 [128, 256] = [Hslice(part), j, B] flattened
        h1T = [None] * T
        c0 = [None] * T
        c1 = [None] * T
        h0n = [None] * T
        h1n = [None] * T
        sig = [[None] * T, [None] * T]
        gt = [[None] * T, [None] * T]
        g0ps = [None] * T
        g1ps = [None] * T
        h0tps = [None] * T
        h1tps = [None] * T
        sigo0 = [None] * T
        tc0x = [None] * T
        tc1x = [None] * T

        bk = [slice(0, 512), slice(512, 1024)]

        def drview(t8):
            return t8.rearrange("p (j m) -> p j m", j=2)

        def emit_g0_aug(t):
            chi = t // CH
            if t % CH == 0:
                if chi + 2 < NCH:
                    load_chunk(chi + 2)
            aug_sl = aug_bufs[chi % 2][:, (t % CH) * B : (t % CH + 1) * B]
            g0 = g0pp.tile([B, G], f32, tag="g0", name=f"g0_{t}")
            g0ps[t] = g0
            last = t == 0
            for nb in range(2):
                mm(g0[:, bk[nb]], aug_sl, w0t_sb[:, bk[nb]], start=True, stop=last)

        def emit_g0_whh(t):
            g0 = g0ps[t]
            hp = h0T[t - 1]
            # bank0 group completes first so sigmoid_if starts earlier
            for nb in range(2):
                for k in range(2):
                    mm(
                        g0[:, bk[nb]],
                        hp[:, k * 128 : (k + 1) * 128],
                        dr_sb["whh0"][:, k, nb * 512 : (nb + 1) * 512],
                        start=False,
                        stop=(k == 1),
                    )

        def emit_g1_bias_ih1(t):
            g1 = g1pp.tile([B, G], f32, tag="g1", name=f"g1_{t}")
            g1ps[t] = g1
            for nb in range(2):
                mm(g1[:, bk[nb]], e0_sb, b1f_sb[:, bk[nb]], start=True, stop=False)
            hp = h0T[t]
            for nb in range(2):
                for k in range(2):
                    mm(
                        g1[:, bk[nb]],
                        hp[:, k * 128 : (k + 1) * 128],
                        dr_sb["wih1"][:, k, nb * 512 : (nb + 1) * 512],
                        start=False,
                        stop=(t == 0 and k == 1),
                    )

        def emit_g1_hh1(t):
            g1 = g1ps[t]
            hq = h1T[t - 1]
            for nb in range(2):
                for k in range(2):
                    mm(
                        g1[:, bk[nb]],
                        hq[:, k * 128 : (k + 1) * 128],
                        dr_sb["whh1"][:, k, nb * 512 : (nb + 1) * 512],
                        start=False,
                        stop=(k == 1),
                    )

        def emit_chain_a0(t):
            gps = g0ps[t]
            s = acts.tile([B, 2 * H], bf16, tag="sigif0", name=f"sigif0_{t}")
            sig[0][t] = s
            nc.scalar.activation(s, gps[:, 0 : 2 * H], Sig, scale=SINV)
            g = acts.tile([B, H], bf16, tag="gt0", name=f"gt0_{t}")
            gt[0][t] = g
            nc.scalar.activation(g, gps[:, 3 * H : G], Tanh, scale=SINV)

        def emit_sigo0(t):
            gps = g0ps[t]
            so = acts.tile([B, H], bf16, tag="sigo0", name=f"sigo0_{t}")
            sigo0[t] = so
            nc.scalar.activation(so, gps[:, 2 * H : 3 * H], Sig, scale=SINV)

        def emit_chain_a1(t):
            gps = g1ps[t]
            s = acts.tile([B, 3 * H], bf16, tag="sig1", name=f"sig1_{t}")
            sig[1][t] = s
            nc.scalar.activation(s, gps[:, 0 : 3 * H], Sig, scale=SINV)
            g = acts.tile([B, H], bf16, tag="gt1", name=f"gt1_{t}")
            gt[1][t] = g
            nc.scalar.activation(g, gps[:, 3 * H : G], Tanh, scale=SINV)

        def emit_cadd(layer, t):
            # c = f*c + i*g ; layer 1 multiplies on GPSIMD (off critical path)
            cp = c0p if layer == 0 else c1p
            cl = c0 if layer == 0 else c1
            feng = nc.vector if layer == 0 else nc.gpsimd
            s = sig[layer][t]
            g = gt[layer][t]
            cn = cp.tile([B, H], bf16, tag=f"c{layer}", name=f"c{layer}_{t}")
            if t == 0:
                nc.vector.tensor_mul(cn, s[:, 0:H], g)
            else:
                fc = acts.tile([B, H], bf16, tag=f"fc{layer}", name=f"fc{layer}_{t}")
                feng.tensor_mul(fc, s[:, H : 2 * H], cl[t - 1])
                ig = acts.tile([B, H], bf16, tag=f"ig{layer}", name=f"ig{layer}_{t}")
                nc.vector.tensor_mul(ig, s[:, 0:H], g)
                nc.vector.tensor_add(cn, ig, fc)
            cl[t] = cn

        def emit_tanhc(layer, t):
            cl = c0 if layer == 0 else c1
            tcx = acts.tile([B, H], bf16, tag=f"tc{layer}", name=f"tc{layer}_{t}")
            nc.scalar.activation(tcx, cl[t], Tanh)
            if layer == 0:
                tc0x[t] = tcx
            else:
                tc1x[t] = tcx

        def emit_h(layer, t, final=False):
            hn = h0n if layer == 0 else h1n
            so = sigo0[t] if layer == 0 else sig[1][t][:, 2 * H : 3 * H]
            tcx = tc0x[t] if layer == 0 else tc1x[t]
            dt = f32 if final else bf16
            h = acts.tile([B, H], dt, tag=f"hn{layer}{'f' if final else ''}",
                          name=f"hn{layer}_{t}")
            nc.vector.tensor_mul(h, so, tcx)
            hn[t] = h

        def emit_hT(layer, t):
            hn = h0n if layer == 0 else h1n
            hTl = h0T if layer == 0 else h1T
            hTpool = h0Tp if layer == 0 else h1Tp
            tps = h0tps if layer == 0 else h1tps
            ps = hTps.tile([128, 256], bf16, tag="htp", name=f"h{layer}tp_{t}")
            nc.tensor.transpose(ps[:, 0:128], hn[t][:, 0:128], identb)
            nc.tensor.transpose(ps[:, 128:256], hn[t][:, 128:256], identb)
            tps[t] = ps
            hsb = hTpool.tile([128, 2 * 128], bf16, tag=f"h{layer}T", name=f"h{layer}T_{t}")
            nc.vector.tensor_copy(
                hsb.bitcast(mybir.dt.uint32), ps.bitcast(mybir.dt.uint32)
            )
            hTl[t] = hsb

        for tau in range(T + 2):
            t0 = tau  # layer-0 step handled this tick
            t1 = tau - 1  # layer-1 step handled this tick
            do0 = t0 < T
            do1 = 0 <= t1 < T
            if tau == 0:
                emit_g0_aug(0)
            if do0:
                if t0 >= 1:
                    emit_g0_whh(t0)
                emit_chain_a0(t0)
            if do1:
                emit_g1_bias_ih1(t1)
                if t1 >= 1:
                    emit_g1_hh1(t1)
            if do0:
                emit_sigo0(t0)
            if do1:
                emit_chain_a1(t1)
            if do0:
                emit_cadd(0, t0)
            if do1:
                emit_cadd(1, t1)
            if do0:
                emit_tanhc(0, t0)
            if do1:
                emit_tanhc(1, t1)
            if t0 + 1 < T:
                emit_g0_aug(t0 + 1)
            if do0:
                emit_h(0, t0)
                emit_hT(0, t0)
            if do1:
                final = t1 == T - 1
                emit_h(1, t1, final=final)
                if not final:
                    emit_hT(1, t1)

        # ------------- final linear: out = h1[T-1] @ Wlin.T + blin -------------
        # h1[T-1] was produced in f32; transpose in f32 for full precision.
        psf = hTps.tile([128, 512], f32, tag="htp", name="hlastT")
        nc.tensor.transpose(psf[:, 0:128], h1n[T - 1][:, 0:128], identf)
        nc.tensor.transpose(psf[:, 256:384], h1n[T - 1][:, 128:256], identf)
        hl = consts.tile([128, H], f32r, tag="hlT")
        nc.vector.tensor_copy(
            hl.rearrange("p (b c) -> p b c", b=2),
            psf.rearrange("p (b c) -> p b c", b=2)[:, :, 0:128],
        )
        outp = hTps.tile([B, P_OUT], f32, tag="htp", name="outp")
        mm(outp, e0_sb, blinf_sb, start=True, stop=False)
        for k in range(2):
            mm(
                outp,
                hl[:, k * 128 : (k + 1) * 128],
                wlint_sb[:, k * P_OUT : (k + 1) * P_OUT],
                start=False,
                stop=(k == 1),
            )
        out_sb = consts.tile([B, P_OUT], f32, tag="outsb")
        nc.vector.tensor_copy(out_sb, outp)
        nc.sync.dma_start(out_d[:, :], out_sb)

    nc.finalize()
    return nc


def _get_module():
    global _MODULE
    if _MODULE is None:
        _MODULE = _build_module()
    return _MODULE


def _dr_layout(w_perm_t):
    # w_perm_t: [H=256, G] (already W[_PERM].T). Returns [128, 2*G] with the
    # two K-chunks side by side: element [p, j*G + n] = w_perm_t[j*128 + p, n].
    return np.ascontiguousarray(
        w_perm_t.reshape(2, 128, G).transpose(1, 0, 2).reshape(128, 2 * G)
    )


def kernel(**inputs):
    global LAST_RESULTS
    from concourse.bass_utils import run_bass_kernel_spmd

    f = lambda a: np.ascontiguousarray(np.asarray(a), dtype=np.float32)
    x = f(inputs["x"])
    emb = f(inputs["emb"])
    Wih0, Whh0 = f(inputs["Wih0"]), f(inputs["Whh0"])
    bih0, bhh0 = f(inputs["bih0"]), f(inputs["bhh0"])
    Wih1, Whh1 = f(inputs["Wih1"]), f(inputs["Whh1"])
    bih1, bhh1 = f(inputs["bih1"]), f(inputs["bhh1"])
    Wlin, blin = f(inputs["Wlin"]), f(inputs["blin"])

    # Fold embedding + biases into layer-0 input weights (scaled by WS).
    w_val = Wih0[:, 0:1]  # [G, 1]
    M0 = Wih0[:, 1 : 1 + D] @ emb.T  # [G, 7]
    b0 = (bih0 + bhh0)[:, None]  # [G, 1]
    W0aug = np.concatenate(
        [w_val, M0, b0, np.zeros((G, 128 - 9), np.float32)], axis=1
)  # [G, 128]

    w0t = np.ascontiguousarray(W0aug[_PERM].T)  # [128, G]
    whh0dr = _dr_layout(np.ascontiguousarray(Whh0[_PERM].T) * WS)
    wih1dr = _dr_layout(np.ascontiguousarray(Wih1[_PERM].T) * WS)
    whh1dr = _dr_layout(np.ascontiguousarray(Whh1[_PERM].T) * WS)
    b1f = np.zeros((128, G), np.float32)
    b1f[0] = (bih1 + bhh1)[_PERM]
    e0 = np.zeros((128, 128), np.float32)
    e0[0] = 1.0
    wlint = np.ascontiguousarray(Wlin.T)  # [H, P_OUT]
    blinf = np.zeros((128, P_OUT), np.float32)
    blinf[0] = blin
    z112 = np.zeros((128 - FA, CH * B), np.float32)

    val = x[:, :, 0]  # [B_FULL, T]
    day = x[:, :, 1].astype(np.int32)  # [B_FULL, T]

    in_maps = []
    for c in range(N_CORES):
        sl = slice(c * B, (c + 1) * B)
        aug = np.zeros((FA, T, B), np.float32)
        aug[0] = val[sl].T
        dT = day[sl].T  # [T, B]
        for d in range(7):
            aug[1 + d] = dT == d
        aug[8] = 1.0
        in_maps.append(
            {
                "aug": np.ascontiguousarray(aug.reshape(FA, T * B)).astype(_BF16),
                "z112": z112.astype(_BF16),
                "w0t": w0t.astype(_BF16),
                "whh0dr": whh0dr,
                "wih1dr": wih1dr,
                "whh1dr": whh1dr,
                "e0": e0.astype(_BF16),
                "b1f": b1f.astype(_BF16),
                "wlint": wlint,
                "blinf": blinf.astype(_BF16),
            }
        )

    res = run_bass_kernel_spmd(_get_module(), in_maps, core_ids=list(range(N_CORES)))
    LAST_RESULTS = res
    out = np.concatenate([r["out"] for r in res.results], axis=0)
    return np.ascontiguousarray(out, dtype=np.float32)
